# revision 33
# baseline (speedup 1.0000x reference)
"""Bass/Trainium2 kernel for BiGraphContrastLayer (GNN message passing).

Computes, for two edge lists (pos/neg) over the same node features:
    h_g = PReLU( D_in^-1/2 A_g D_out^-1/2 feats @ W + b )
returning stack([h_pos, h_neg]) of shape [2, N, Dout].

Strategy (8 NeuronCores, SPMD, no collectives). Using linearity,
    (D_in^-1/2 A D_out^-1/2 feats) @ W = D_in^-1/2 (A (D_out^-1/2 feats)) @ W
so the device aggregates raw (host-prescaled) feature rows FIRST and applies
W once per destination tile afterwards — there is no feats@W precompute
phase and no intermediate DRAM round trip at all:

  Host: x_g = f16(ns_g * feats)  (per-graph out-degree prescale), edges
  bucketed by dst tile, dst tiles dealt to cores (prefix-balanced so all 8
  cores share one instruction stream), edges packed into 128-slot chunks at
  supergroup x bank granularity (bank = 32K-row window for int16 gather
  indices; padding ~2%).

  Device, per (graph, supergroup) job:
    dma_gather pulls x[src] rows for each bank region into SBUF; per dst
    tile, one-hot matmuls (lhsT = gathered rows, rhs = is_equal(iota, off))
    segment-sum into a TRANSPOSED PSUM accumulator aggT[d, j]; ScalarE
    copies aggT to SBUF f16; one matmul aggT^T @ W -> h[j, d']; ScalarE
    PReLU with the in-degree norm nd folded into the activation scale
    (prelu(s*x) = s*prelu(x) for s>=0). f16 output, host upcasts.
"""

import math
import tempfile
from dataclasses import dataclass

import numpy as np

P = 128   # partitions
D = 128   # feature dim (Din == Dout == 128)
NBANK = 4
QUAD = 4  # dst-tile positions sharing one [P, 512] PSUM bank


# --------------------------------------------------------------------------
# Config
# --------------------------------------------------------------------------
@dataclass
class Config:
    n_nodes: int = 100000
    n_cores: int = 8
    sg: int = 20       # dst-tile positions per supergroup
    oh_mod: int = 3    # every oh_mod-th one-hot build goes to GpSimd (0=off)
    act_prelu: bool = True   # final PReLU on ScalarE (not in CoreSim)
    idx16: bool = False      # 16-partition idx load (real HW needs all)
    exact_ni: bool = True    # exact gather num_idxs on pre-written buffers
                             # (sound on HW; CoreSim NaN-inits fresh tiles)
    gbufs: int = 2           # gather buffer count

    @property
    def t_global(self) -> int:
        return math.ceil(self.n_nodes / P)

    @property
    def n_pad(self) -> int:
        return self.t_global * P

    @property
    def t_core(self) -> int:
        return math.ceil(self.t_global / self.n_cores)

    @property
    def bank_tiles(self) -> int:
        return math.ceil(self.t_global / NBANK)

    @property
    def bank_rows(self) -> int:
        return self.bank_tiles * P


# --------------------------------------------------------------------------
# Host-side preprocessing
# --------------------------------------------------------------------------
def _deal_tiles(bcnt, cfg: Config):
    """Deal tiles to cores with PER-BANK prefix balancing: sort tiles by
    total count desc; per group of n_cores, assign tiles (largest first) to
    the core whose per-bank running totals would deviate least from the
    group mean. Keeps each core's per-bank slot prefixes aligned so the
    shared (union) build structure has minimal slack.
    bcnt: [t_global, NBANK] per-tile per-bank edge counts.
    Returns core_tiles [n_cores, t_core] (-1 = null)."""
    nc, tc = cfg.n_cores, cfg.t_core
    tot = bcnt.sum(axis=1)
    order = np.argsort(-tot, kind="stable")
    core_tiles = np.full((nc, tc), -1, np.int64)
    cum = np.zeros((nc, NBANK), np.float64)
    for k in range(tc):
        grp = order[k * nc: (k + 1) * nc]
        taken = np.zeros(nc, bool)
        for t in grp:                       # biggest tile first
            best, bcost = -1, None
            newmean = (cum.sum(0) + bcnt[t]) / nc
            for c in range(nc):
                if taken[c]:
                    continue
                cost = float(((cum[c] + bcnt[t] - newmean) ** 2).sum())
                if bcost is None or cost < bcost:
                    best, bcost = c, cost
            taken[best] = True
            core_tiles[best, k] = t
            cum[best] += bcnt[t]
    return core_tiles


def _sg_split(tc, sg):
    """Split positions into supergroups of ~sg, with a tapered tail so the
    final jobs are small (shrinks the compute-only tail after the last
    gather)."""
    sizes = []
    rem = tc
    while rem > 2 * sg:
        sizes.append(sg)
        rem -= sg
    while rem > 4:
        piece = -(-rem // 2)
        sizes.append(piece)
        rem -= piece
    if rem:
        sizes.append(rem)
    out = []
    k0 = 0
    for s in sizes:
        out.append((k0, s))
        k0 += s
    return out


def _layout_graph(src, dst, core_tiles, cfg: Config):
    """Shared chunk/build layout for one graph + per-core idx/off data.

    Returns dict:
      sgs:    list of (k0, kn)
      C:      [n_sg, NBANK] shared chunk counts
      M:      [n_sg, NBANK] true max slots per bank region (<= C*128)
      gbase:  [n_sg, NBANK] chunk offset of bank region within the sg tile
      builds: per sg, list of (q0, gcol, col, wbase_tl, wtiles, start, stop);
              col is global off-column index; gcol is chunk index within the
              sg gather tile; window = quad-local tiles [wbase_tl, +wtiles)
      n_cols: total off columns
      idx:    per-core [P, total_chunks*8] int16 (wrapped gather indices)
      off:    per-core [P, n_cols] f32
      nseg:   total chunks (sum of C)
    """
    ncores, tc = cfg.n_cores, cfg.t_core
    sgs = _sg_split(tc, cfg.sg)
    n_sg = len(sgs)
    brows = cfg.bank_rows

    # per-core, per-tile-position, per-bank edge lists (rows, offs)
    tile_edges = [[None] * tc for _ in range(ncores)]
    order = np.argsort(dst, kind="stable")
    src_s = src[order]
    dst_s = dst[order]
    tstart = np.zeros(cfg.t_global + 1, np.int64)
    np.cumsum(np.bincount(dst_s // P, minlength=cfg.t_global), out=tstart[1:])
    for c in range(ncores):
        for k in range(tc):
            t = core_tiles[c, k]
            if t < 0:
                tile_edges[c][k] = None
                continue
            e0, e1 = int(tstart[t]), int(tstart[t + 1])
            rows = src_s[e0:e1]
            offs = (dst_s[e0:e1] % P).astype(np.int64)
            bank = rows // brows
            bo = np.argsort(bank, kind="stable")
            rows, offs, bank = rows[bo], offs[bo], bank[bo]
            bcut = np.searchsorted(bank, np.arange(NBANK + 1))
            tile_edges[c][k] = (rows, offs, bcut)

    C = np.zeros((n_sg, NBANK), np.int64)
    M = np.zeros((n_sg, NBANK), np.int64)   # true max slots (<= C*128)
    gbase = np.zeros((n_sg, NBANK), np.int64)
    seg = []  # per sg, per bank: per core: list of (k, s0, s1) slot ranges
    for si, (k0, kn) in enumerate(sgs):
        for b in range(NBANK):
            percore = []
            maxm = 0
            for c in range(ncores):
                pos = 0
                rl = []
                for ki in range(kn):
                    te = tile_edges[c][k0 + ki]
                    if te is None:
                        rl.append((ki, pos, pos))
                        continue
                    n = int(te[2][b + 1] - te[2][b])
                    rl.append((ki, pos, pos + n))
                    pos += n
                percore.append(rl)
                maxm = max(maxm, pos)
            C[si, b] = -(-maxm // 128)
            M[si, b] = maxm
            seg.append(percore)
        C[si, 0] = max(C[si, 0], 1)  # dummy-build anchor
        M[si, 0] = max(M[si, 0], 1)
        gbase[si] = np.concatenate([[0], np.cumsum(C[si])[:-1]])

    # shared build list at QUAD granularity: 4 consecutive tile positions
    # share one [P, 512] PSUM bank; a build covers one chunk's intersection
    # with one quad, with a column window spanning the (cross-core union of)
    # tiles it touches. First/last build per quad use the full quad window
    # so accumulate start/stop flags are uniform.
    # build entry: (q0, gcol, col, wbase_tl, wtiles, start, stop)
    builds = []
    n_cols = 0
    for si, (k0, kn) in enumerate(sgs):
        bl = []
        for q0 in range(0, kn, QUAD):
            qn = min(QUAD, kn - q0)
            ent = []  # (b, ch, tl_min, tl_max)
            for b in range(NBANK):
                percore = seg[si * NBANK + b]
                # per chunk: union tile window
                win = {}
                for c in range(ncores):
                    for ki in range(q0, q0 + qn):
                        _, s0, s1 = percore[c][ki]
                        if s1 <= s0:
                            continue
                        for ch in range(s0 // 128, -(-s1 // 128)):
                            tl = ki - q0
                            if ch in win:
                                a, z = win[ch]
                                win[ch] = (min(a, tl), max(z, tl))
                            else:
                                win[ch] = (tl, tl)
                for ch in sorted(win):
                    a, z = win[ch]
                    ent.append((b, ch, a, z))
            if not ent:
                ent.append((0, 0, 0, qn - 1))  # dummy: zeroes the psum
            nb = len(ent)
            for j, (b, ch, a, z) in enumerate(ent):
                first = j == 0
                last = j == nb - 1
                if first or last:
                    a, z = 0, qn - 1  # full quad window
                bl.append((q0, int(gbase[si, b] + ch), n_cols, a, z - a + 1,
                           first, last))
                n_cols += 1
        builds.append(bl)

    # per-core arrays
    total_chunks = int(C.sum())
    idx_all = np.zeros((ncores, P, total_chunks * 8), np.int16)
    off_all = np.full((ncores, P, n_cols), 512.0, np.float32)
    # chunk column base per (si, b)
    cb = np.concatenate([[0], np.cumsum(C.reshape(-1))[:-1]]).reshape(
        n_sg, NBANK)
    for si, (k0, kn) in enumerate(sgs):
        for b in range(NBANK):
            nslot = int(C[si, b]) * 128
            if nslot == 0:
                continue
            percore = seg[si * NBANK + b]
            for c in range(ncores):
                rows = np.zeros(nslot, np.int64)
                offs = np.full(nslot, 512.0, np.float32)
                ktag = np.full(nslot, -1, np.int64)
                for (ki, s0, s1) in percore[c]:
                    if s1 == s0:
                        continue
                    te = tile_edges[c][k0 + ki]
                    e0, e1 = int(te[2][b]), int(te[2][b + 1])
                    rows[s0:s1] = te[0][e0:e1] - b * brows
                    offs[s0:s1] = te[1][e0:e1]
                    ktag[s0:s1] = ki
                blk = rows.astype(np.int16).reshape(-1, 16).T  # [16, n/16]
                c0 = int(cb[si, b])
                idx_all[c, :, c0 * 8: c0 * 8 + nslot // 16] = np.tile(
                    blk, (8, 1))
                # off columns for builds of this bank
                for (q0, gcol, col, a, w, _s, _e) in builds[si]:
                    ch = gcol - int(gbase[si, b])
                    if not (0 <= ch < int(C[si, b])):
                        continue
                    sl = slice(ch * 128, (ch + 1) * 128)
                    kt = ktag[sl]
                    inw = (kt >= q0 + a) & (kt < q0 + a + w)
                    off_all[c, :, col] = np.where(
                        inw, (kt - (q0 + a)) * 128 + offs[sl], 512.0)

    return dict(sgs=sgs, C=C, M=M, gbase=gbase, builds=builds, n_cols=n_cols,
                idx=idx_all, off=off_all, nseg=total_chunks)


def preprocess(feats, W, b, prelu_a, src_pos, dst_pos, src_neg, dst_neg,
               cfg: Config):
    n, ncores, tc = cfg.n_nodes, cfg.n_cores, cfg.t_core
    feats = np.asarray(feats, np.float32)
    W = np.asarray(W, np.float32)
    b = np.asarray(b, np.float32)
    prelu_a = np.asarray(prelu_a, np.float32)

    xs, plans, layouts, nds = [], [], [], []
    for src, dst in ((src_pos, dst_pos), (src_neg, dst_neg)):
        src = np.asarray(src, np.int64)
        dst = np.asarray(dst, np.int64)
        dego = np.bincount(src, minlength=n).astype(np.float64)
        degi = np.bincount(dst, minlength=n).astype(np.float64)
        ns = np.where(dego > 0, 1.0 / np.sqrt(np.maximum(dego, 1.0)), 0.0)
        nd = np.where(degi > 0, 1.0 / np.sqrt(np.maximum(degi, 1.0)), 0.0)
        x = np.zeros((cfg.n_pad, D), np.float16)
        x[:n] = (feats * ns[:, None].astype(np.float32)).astype(np.float16)
        xs.append(x)
        nds.append(nd.astype(np.float32))
        bank = src // cfg.bank_rows
        bcnt = np.zeros((cfg.t_global, NBANK), np.int64)
        np.add.at(bcnt, ((dst // P), bank), 1)
        ct = _deal_tiles(bcnt, cfg)
        plans.append(dict(core_tiles=ct))
        layouts.append(_layout_graph(src, dst, ct, cfg))

    # nd per (graph, position, core): [ncores, P, 2*t_core] f32
    nd_arr = np.zeros((ncores, P, 2 * tc), np.float32)
    for g in range(2):
        ndpad = np.zeros(cfg.n_pad, np.float32)
        ndpad[:n] = nds[g]
        ndt = ndpad.reshape(cfg.t_global, P).T
        ct = plans[g]["core_tiles"]
        for c in range(ncores):
            valid = ct[c] >= 0
            nd_arr[c][:, g * tc: (g + 1) * tc][:, valid] = ndt[:, ct[c][valid]]

    iota = np.tile(np.arange(QUAD * P, dtype=np.float32),
                   (P, 1)).astype(np.float16)
    a_rep = np.full((P, 1), float(prelu_a.reshape(-1)[0]), np.float32)
    b_rep = np.tile(b.reshape(1, D), (P, 1)).astype(np.float32)

    in_maps = []
    for c in range(ncores):
        in_maps.append({
            "x0": xs[0], "x1": xs[1],
            "w_in": W, "a_rep": a_rep, "b_rep": b_rep,
            "nd_in": nd_arr[c],
            "idx_in": np.concatenate(
                [layouts[0]["idx"][c], layouts[1]["idx"][c]], axis=1),
            "off_in": np.concatenate(
                [layouts[0]["off"][c], layouts[1]["off"][c]], axis=1),
            "iota_in": iota,
        })
    meta = {
        "layouts": layouts,
        "use_bias": bool(np.any(b != 0.0)),
    }
    return in_maps, plans, meta


# --------------------------------------------------------------------------
# Device kernel builder
# --------------------------------------------------------------------------
def build_kernel(nc, tc, cfg: Config, meta):
    from contextlib import ExitStack

    import concourse.mybir as mybir

    f32 = mybir.dt.float32
    f16 = mybir.dt.float16
    i16 = mybir.dt.int16
    Alu = mybir.AluOpType
    Act = mybir.ActivationFunctionType

    tcn, npad = cfg.t_core, cfg.n_pad
    layouts = meta["layouts"]
    use_bias = meta["use_bias"]
    nseg = [layouts[g]["nseg"] for g in range(2)]
    ncols = [layouts[g]["n_cols"] for g in range(2)]
    # max chunks/cols per supergroup (for fixed-size pool tiles)
    sg_chunks = []
    sg_cols = []
    for g in range(2):
        for si in range(len(layouts[g]["sgs"])):
            sg_chunks.append(int(layouts[g]["C"][si].sum()))
            sg_cols.append(len(layouts[g]["builds"][si]))
    cmax = max(sg_chunks)
    colmax = max(sg_cols)
    knmax = max(kn for g in range(2) for (_k0, kn) in layouts[g]["sgs"])

    x_dram = [nc.dram_tensor(f"x{g}", [npad, D], f16, kind="ExternalInput").ap()
              for g in range(2)]
    w_in = nc.dram_tensor("w_in", [P, D], f32, kind="ExternalInput").ap()
    a_rep = nc.dram_tensor("a_rep", [P, 1], f32, kind="ExternalInput").ap()
    b_rep = nc.dram_tensor("b_rep", [P, D], f32, kind="ExternalInput").ap()
    nd_in = nc.dram_tensor("nd_in", [P, 2 * tcn], f32, kind="ExternalInput").ap()
    idx_in = nc.dram_tensor("idx_in", [P, 8 * sum(nseg)], i16,
                            kind="ExternalInput").ap()
    off_in = nc.dram_tensor("off_in", [P, sum(ncols)], f32,
                            kind="ExternalInput").ap()
    iota_in = nc.dram_tensor("iota_in", [P, QUAD * P], f16,
                             kind="ExternalInput").ap()
    out = nc.dram_tensor("out", [2, P, tcn, D], f16, kind="ExternalOutput").ap()

    pb = dict(gpool=cfg.gbufs, ipool=2, opool=2, ohpool=32, apool=6,
              spool=2, tpool=4, ppool=4, hpool=4)
    pb.update(globals().get("POOL_BUFS") or {})

    with ExitStack() as ctx:
        const = ctx.enter_context(tc.tile_pool(name="const", bufs=1))
        gpool = ctx.enter_context(tc.tile_pool(name="gpool", bufs=pb["gpool"]))
        ipool = ctx.enter_context(tc.tile_pool(name="ipool", bufs=pb["ipool"]))
        opool = ctx.enter_context(tc.tile_pool(name="opool", bufs=pb["opool"]))
        ohpool = ctx.enter_context(tc.tile_pool(name="ohpool",
                                                bufs=pb["ohpool"]))
        apool = ctx.enter_context(tc.tile_pool(name="apool", bufs=pb["apool"]))
        spool = ctx.enter_context(tc.tile_pool(name="spool", bufs=pb["spool"]))
        tpool = ctx.enter_context(tc.tile_pool(name="tpool", bufs=pb["tpool"]))
        ppool = ctx.enter_context(tc.tile_pool(name="ppool", bufs=pb["ppool"],
                                               space="PSUM"))
        hpool = ctx.enter_context(tc.tile_pool(name="hpool", bufs=pb["hpool"],
                                               space="PSUM"))

        idx_base = [0, 8 * nseg[0]]
        col_base = [0, ncols[0]]
        # job list: interleave the two graphs' supergroups
        jobs = []
        for si in range(max(len(layouts[0]["sgs"]), len(layouts[1]["sgs"]))):
            for g in range(2):
                if si < len(layouts[g]["sgs"]):
                    jobs.append((g, si))

        # running chunk/col offsets per graph
        coff = [np.concatenate([[0], np.cumsum(
            layouts[g]["C"].reshape(-1))]).astype(int) for g in range(2)]
        boff = [np.concatenate([[0], np.cumsum(
            [len(bl) for bl in layouts[g]["builds"]])]).astype(int)
            for g in range(2)]
        # slot coverage written by each gather buffer's first-use job; later
        # jobs on the same buffer may use exact (unpadded) num_idxs only
        # where their region was fully pre-written (masked stale slots must
        # hold finite f16 data, not uninitialized SBUF).
        buf_cover = [0] * pb["gpool"]

        ipart = 16 if cfg.idx16 else P

        def issue_loads(jidx, g, si):
            lay = layouts[g]
            Crow = lay["C"][si]
            nch = int(Crow.sum())
            c0 = int(coff[g][si * NBANK])
            bl = lay["builds"][si]
            col0 = int(boff[g][si])
            it = ipool.tile([P, cmax * 8], i16, tag="gidx")
            nc.sync.dma_start(
                out=it[:ipart, : nch * 8],
                in_=idx_in[:ipart, idx_base[g] + c0 * 8:
                           idx_base[g] + (c0 + nch) * 8])
            ot = opool.tile([P, colmax], f32, tag="goff")
            nc.sync.dma_start(
                out=ot[:, : len(bl)],
                in_=off_in[:, col_base[g] + col0:
                           col_base[g] + col0 + len(bl)])
            gt = gpool.tile([P, cmax, D], f16, tag="gather")
            first_use = jidx < pb["gpool"]
            exact_ok = (cfg.exact_ni and not first_use
                        and nch <= buf_cover[jidx % pb["gpool"]])
            if first_use:
                buf_cover[jidx % pb["gpool"]] = nch
            for b in range(NBANK):
                Cb = int(Crow[b])
                if Cb == 0:
                    continue
                lo = int(lay["gbase"][si, b])
                rows = min(cfg.bank_rows, npad - b * cfg.bank_rows)
                ni = int(lay["M"][si, b]) if exact_ok else Cb * P
                icols = -(-ni // 16)
                nc.gpsimd.dma_gather(
                    out_ap=gt[:, lo: lo + Cb, :],
                    in_ap=x_dram[g][b * cfg.bank_rows:
                                    b * cfg.bank_rows + rows, :],
                    idxs_ap=it[:, lo * 8: lo * 8 + icols],
                    num_idxs=ni, num_idxs_reg=ni,
                    elem_size=D, single_packet=False)
            return it, ot, gt

        if cfg.idx16:
            # idx DMAs only write partitions 0..15 (all the gather reads);
            # zero the idx buffers once so the rest is initialized.
            for _ in range(pb["ipool"]):
                zt = ipool.tile([P, cmax * 8], i16, tag="gidx")
                nc.vector.memset(zt[:], 0)

        # job 0's loads + gathers go first so the DMA stream starts
        # immediately; constants follow (first needed ~10us in).
        pre = {0: issue_loads(0, *jobs[0])}

        # ---- constants ----
        iota_sb = const.tile([P, QUAD * P], f16)
        nc.sync.dma_start(out=iota_sb[:], in_=iota_in)
        w_sb = const.tile([P, D], f16)
        nc.gpsimd.dma_start(out=w_sb[:], in_=w_in)  # f32 -> f16 cast DMA
        a_sb = const.tile([P, 1], f32)
        nc.sync.dma_start(out=a_sb[:], in_=a_rep)
        nd_sb = const.tile([P, 2 * tcn], f32)
        nc.sync.dma_start(out=nd_sb[:], in_=nd_in)
        if use_bias:
            b_sb = const.tile([P, D], f32)
            nc.sync.dma_start(out=b_sb[:], in_=b_rep)

        obuild = 0  # global one-hot build counter for engine assignment
        for jidx, (g, si) in enumerate(jobs):
            lay = layouts[g]
            (k0, kn) = lay["sgs"][si]
            bl = lay["builds"][si]
            col0 = int(boff[g][si])
            it, ot, gt = pre.pop(jidx, None) or issue_loads(jidx, g, si)

            stg = spool.tile([P, knmax, D], f16, tag="stg")
            # group builds by quad
            by_q = {}
            for (q0, gcol, col, a, w, s, e) in bl:
                by_q.setdefault(q0, []).append((gcol, col, a, w, s, e))
            for q0 in sorted(by_q):
                qn = min(QUAD, kn - q0)
                ps = ppool.tile([P, QUAD * D], f32)
                for (gcol, col, a, w, s, e) in by_q[q0]:
                    oh = ohpool.tile([P, QUAD * P], f16)
                    eng = nc.vector
                    if w == 1 and cfg.oh_mod and (obuild % cfg.oh_mod == 0):
                        eng = nc.gpsimd
                    obuild += 1
                    eng.tensor_scalar(
                        out=oh[:, : w * P], in0=iota_sb[:, : w * P],
                        scalar1=ot[:, col - col0: col - col0 + 1],
                        scalar2=None, op0=Alu.is_equal)
                    nc.tensor.matmul(
                        out=ps[:, a * D: (a + w) * D],
                        lhsT=gt[:, gcol, :], rhs=oh[:, : w * P],
                        start=s, stop=e)
                # aggT (psum, [d, quad*j]) -> SBUF f16, one copy per quad
                at = apool.tile([P, QUAD * D], f16, tag="aggT")
                nc.scalar.activation(out=at[:, : qn * D],
                                     in_=ps[:, : qn * D], func=Act.Copy)
                for ki in range(q0, q0 + qn):
                    tl = ki - q0
                    hp = hpool.tile([P, D], f32)
                    nc.tensor.matmul(out=hp[:], lhsT=at[:, tl * D: (tl + 1) * D],
                                     rhs=w_sb[:], start=True, stop=True)
                    kslot = g * tcn + (k0 + ki)
                    if cfg.act_prelu and not use_bias:
                        nc.scalar.activation(
                            out=stg[:, ki, :], in_=hp[:], func=Act.Prelu,
                            scale=nd_sb[:, kslot: kslot + 1],
                            alpha=a_sb[:, :1])
                    else:
                        h1 = tpool.tile([P, D], f32, tag="h1")
                        nc.vector.tensor_scalar(
                            out=h1[:], in0=hp[:],
                            scalar1=nd_sb[:, kslot: kslot + 1],
                            scalar2=None, op0=Alu.mult)
                        if use_bias:
                            h2 = tpool.tile([P, D], f32, tag="h2")
                            nc.vector.tensor_tensor(out=h2[:], in0=h1[:],
                                                    in1=b_sb[:], op=Alu.add)
                            h1 = h2
                        neg = tpool.tile([P, D], f32, tag="neg")
                        nc.vector.tensor_scalar(
                            out=neg[:], in0=h1[:], scalar1=0.0,
                            scalar2=a_sb[:, :1], op0=Alu.min, op1=Alu.mult)
                        pos = tpool.tile([P, D], f32, tag="pos")
                        nc.vector.tensor_scalar(
                            out=pos[:], in0=h1[:], scalar1=0.0,
                            scalar2=None, op0=Alu.max)
                        nc.vector.tensor_tensor(out=stg[:, ki, :], in0=neg[:],
                                                in1=pos[:], op=Alu.add)
            nc.sync.dma_start(out=out[g, :, k0: k0 + kn, :],
                              in_=stg[:, :kn, :])
    return out


# --------------------------------------------------------------------------
# Driver
# --------------------------------------------------------------------------
def _build_program(cfg: Config, meta):
    import concourse.bacc as bacc
    import concourse.tile as tile

    nc = bacc.Bacc("TRN2", target_bir_lowering=False, debug=False,
                   enable_asserts=False, num_devices=cfg.n_cores)
    with tile.TileContext(nc) as tc:
        build_kernel(nc, tc, cfg, meta)
    nc.compile()
    return nc


def _unscramble(results, plans, cfg: Config):
    n = cfg.n_nodes
    full = np.zeros((2, n, D), np.float32)
    for g in range(2):
        ct_all = plans[g]["core_tiles"]
        for core in range(cfg.n_cores):
            oc = np.asarray(results[core]["out"], np.float32)  # [2,P,tc,D]
            for k in range(cfg.t_core):
                t = int(ct_all[core, k])
                if t < 0:
                    continue
                r0 = t * P
                r1 = min(r0 + P, n)
                full[g, r0:r1] = oc[g, : r1 - r0, k, :]
    return full


_PROGRAM_CACHE = {}


def _meta_key(cfg: Config, meta):
    import hashlib
    hsh = hashlib.sha256()
    for g in range(2):
        lay = meta["layouts"][g]
        hsh.update(lay["C"].tobytes())
        for bl in lay["builds"]:
            hsh.update(np.asarray(bl, np.int64).tobytes())
    return (cfg.n_nodes, cfg.n_cores, cfg.sg, cfg.oh_mod, cfg.act_prelu,
            cfg.gbufs, cfg.idx16, cfg.exact_ni, meta["use_bias"],
            hsh.hexdigest())


def run(inputs, cfg: Config, trace=False):
    from concourse.bass_utils import run_bass_kernel_spmd

    in_maps, plans, meta = preprocess(
        inputs["feats"], inputs["W"], inputs["b"], inputs["prelu_a"],
        inputs["src_pos"], inputs["dst_pos"],
        inputs["src_neg"], inputs["dst_neg"], cfg)

    key = _meta_key(cfg, meta)
    nc = _PROGRAM_CACHE.get(key)
    if nc is None:
        nc = _build_program(cfg, meta)
        _PROGRAM_CACHE[key] = nc

    kwargs = {}
    if trace:
        kwargs = dict(trace=True, tmpdir=tempfile.mkdtemp(prefix="bgc_trace_"))
    try:
        res = run_bass_kernel_spmd(nc, in_maps,
                                   core_ids=list(range(cfg.n_cores)), **kwargs)
    except Exception:
        # Intermittent NRT_EXEC_UNIT_UNRECOVERABLE wedge under the axon
        # tunnel; a retry (with core reset requested) usually clears it.
        import os
        os.environ["NEURON_RT_RESET_CORES"] = "1"
        res = run_bass_kernel_spmd(nc, in_maps,
                                   core_ids=list(range(cfg.n_cores)), **kwargs)
    full = _unscramble(res.results, plans, cfg)
    return full, res


def kernel(**inputs) -> np.ndarray:
    cfg = Config()
    full, _ = run(inputs, cfg)
    return full


# revision 37
# speedup vs baseline: 1.0032x; 1.0032x over previous
"""Bass/Trainium2 kernel for BiGraphContrastLayer (GNN message passing).

Computes, for two edge lists (pos/neg) over the same node features:
    h_g = PReLU( D_in^-1/2 A_g D_out^-1/2 feats @ W + b )
returning stack([h_pos, h_neg]) of shape [2, N, Dout].

Strategy (8 NeuronCores, SPMD, no collectives). Using linearity,
    (D_in^-1/2 A D_out^-1/2 feats) @ W = D_in^-1/2 (A (D_out^-1/2 feats)) @ W
so the device aggregates raw (host-prescaled) feature rows FIRST and applies
W once per destination tile afterwards — there is no feats@W precompute
phase and no intermediate DRAM round trip at all:

  Host: x_g = f16(ns_g * feats)  (per-graph out-degree prescale), edges
  bucketed by dst tile, dst tiles dealt to cores (prefix-balanced so all 8
  cores share one instruction stream), edges packed into 128-slot chunks at
  supergroup x bank granularity (bank = 32K-row window for int16 gather
  indices; padding ~2%).

  Device, per (graph, supergroup) job:
    dma_gather pulls x[src] rows for each bank region into SBUF; per dst
    tile, one-hot matmuls (lhsT = gathered rows, rhs = is_equal(iota, off))
    segment-sum into a TRANSPOSED PSUM accumulator aggT[d, j]; ScalarE
    copies aggT to SBUF f16; one matmul aggT^T @ W -> h[j, d']; ScalarE
    PReLU with the in-degree norm nd folded into the activation scale
    (prelu(s*x) = s*prelu(x) for s>=0). f16 output, host upcasts.
"""

import math
import tempfile
from dataclasses import dataclass

import numpy as np

P = 128   # partitions
D = 128   # feature dim (Din == Dout == 128)
NBANK = 4
QUAD = 4  # dst-tile positions sharing one [P, 512] PSUM bank


# --------------------------------------------------------------------------
# Config
# --------------------------------------------------------------------------
@dataclass
class Config:
    n_nodes: int = 100000
    n_cores: int = 8
    sg: int = 20       # dst-tile positions per supergroup
    oh_mod: int = 4    # every oh_mod-th one-hot build goes to GpSimd (0=off)
    act_prelu: bool = True   # final PReLU on ScalarE (not in CoreSim)
    idx16: bool = False      # 16-partition idx load (real HW needs all)
    exact_ni: bool = True    # exact gather num_idxs on pre-written buffers
                             # (sound on HW; CoreSim NaN-inits fresh tiles)
    gbufs: int = 2           # gather buffer count
    pool_cut: int = 0        # last N jobs: no Pool one-hot builds
    ghead: int = 0           # split head chunks off job-0 bank-0 gather

    @property
    def t_global(self) -> int:
        return math.ceil(self.n_nodes / P)

    @property
    def n_pad(self) -> int:
        return self.t_global * P

    @property
    def t_core(self) -> int:
        return math.ceil(self.t_global / self.n_cores)

    @property
    def bank_tiles(self) -> int:
        return math.ceil(self.t_global / NBANK)

    @property
    def bank_rows(self) -> int:
        return self.bank_tiles * P


# --------------------------------------------------------------------------
# Host-side preprocessing
# --------------------------------------------------------------------------
def _deal_tiles(bcnt, cfg: Config):
    """Deal tiles to cores with PER-BANK prefix balancing: sort tiles by
    total count desc; per group of n_cores, assign tiles (largest first) to
    the core whose per-bank running totals would deviate least from the
    group mean. Keeps each core's per-bank slot prefixes aligned so the
    shared (union) build structure has minimal slack.
    bcnt: [t_global, NBANK] per-tile per-bank edge counts.
    Returns core_tiles [n_cores, t_core] (-1 = null)."""
    nc, tc = cfg.n_cores, cfg.t_core
    tot = bcnt.sum(axis=1)
    order = np.argsort(-tot, kind="stable")
    core_tiles = np.full((nc, tc), -1, np.int64)
    cum = np.zeros((nc, NBANK), np.float64)
    for k in range(tc):
        grp = order[k * nc: (k + 1) * nc]
        taken = np.zeros(nc, bool)
        for t in grp:                       # biggest tile first
            best, bcost = -1, None
            newmean = (cum.sum(0) + bcnt[t]) / nc
            for c in range(nc):
                if taken[c]:
                    continue
                cost = float(((cum[c] + bcnt[t] - newmean) ** 2).sum())
                if bcost is None or cost < bcost:
                    best, bcost = c, cost
            taken[best] = True
            core_tiles[best, k] = t
            cum[best] += bcnt[t]
    return core_tiles


def _sg_split(tc, sg):
    """Split positions into supergroups of ~sg, with a tapered tail so the
    final jobs are small (shrinks the compute-only tail after the last
    gather)."""
    sizes = []
    rem = tc
    while rem > 2 * sg:
        sizes.append(sg)
        rem -= sg
    while rem > 4:
        piece = -(-rem // 2)
        sizes.append(piece)
        rem -= piece
    if rem:
        sizes.append(rem)
    out = []
    k0 = 0
    for s in sizes:
        out.append((k0, s))
        k0 += s
    return out


def _layout_graph(src, dst, core_tiles, cfg: Config):
    """Shared chunk/build layout for one graph + per-core idx/off data.

    Returns dict:
      sgs:    list of (k0, kn)
      C:      [n_sg, NBANK] shared chunk counts
      M:      [n_sg, NBANK] true max slots per bank region (<= C*128)
      gbase:  [n_sg, NBANK] chunk offset of bank region within the sg tile
      builds: per sg, list of (q0, gcol, col, wbase_tl, wtiles, start, stop);
              col is global off-column index; gcol is chunk index within the
              sg gather tile; window = quad-local tiles [wbase_tl, +wtiles)
      n_cols: total off columns
      idx:    per-core [P, total_chunks*8] int16 (wrapped gather indices)
      off:    per-core [P, n_cols] f32
      nseg:   total chunks (sum of C)
    """
    ncores, tc = cfg.n_cores, cfg.t_core
    sgs = _sg_split(tc, cfg.sg)
    n_sg = len(sgs)
    brows = cfg.bank_rows

    # per-core, per-tile-position, per-bank edge lists (rows, offs)
    tile_edges = [[None] * tc for _ in range(ncores)]
    order = np.argsort(dst, kind="stable")
    src_s = src[order]
    dst_s = dst[order]
    tstart = np.zeros(cfg.t_global + 1, np.int64)
    np.cumsum(np.bincount(dst_s // P, minlength=cfg.t_global), out=tstart[1:])
    for c in range(ncores):
        for k in range(tc):
            t = core_tiles[c, k]
            if t < 0:
                tile_edges[c][k] = None
                continue
            e0, e1 = int(tstart[t]), int(tstart[t + 1])
            rows = src_s[e0:e1]
            offs = (dst_s[e0:e1] % P).astype(np.int64)
            bank = rows // brows
            bo = np.argsort(bank, kind="stable")
            rows, offs, bank = rows[bo], offs[bo], bank[bo]
            bcut = np.searchsorted(bank, np.arange(NBANK + 1))
            tile_edges[c][k] = (rows, offs, bcut)

    C = np.zeros((n_sg, NBANK), np.int64)
    M = np.zeros((n_sg, NBANK), np.int64)   # true max slots (<= C*128)
    gbase = np.zeros((n_sg, NBANK), np.int64)
    seg = []  # per sg, per bank: per core: list of (k, s0, s1) slot ranges
    for si, (k0, kn) in enumerate(sgs):
        for b in range(NBANK):
            percore = []
            maxm = 0
            for c in range(ncores):
                pos = 0
                rl = []
                for ki in range(kn):
                    te = tile_edges[c][k0 + ki]
                    if te is None:
                        rl.append((ki, pos, pos))
                        continue
                    n = int(te[2][b + 1] - te[2][b])
                    rl.append((ki, pos, pos + n))
                    pos += n
                percore.append(rl)
                maxm = max(maxm, pos)
            C[si, b] = -(-maxm // 128)
            M[si, b] = maxm
            seg.append(percore)
        C[si, 0] = max(C[si, 0], 1)  # dummy-build anchor
        M[si, 0] = max(M[si, 0], 1)
        gbase[si] = np.concatenate([[0], np.cumsum(C[si])[:-1]])

    # shared build list at QUAD granularity: 4 consecutive tile positions
    # share one [P, 512] PSUM bank; a build covers one chunk's intersection
    # with one quad, with a column window spanning the (cross-core union of)
    # tiles it touches. First/last build per quad use the full quad window
    # so accumulate start/stop flags are uniform.
    # build entry: (q0, gcol, col, wbase_tl, wtiles, start, stop)
    builds = []
    n_cols = 0
    for si, (k0, kn) in enumerate(sgs):
        bl = []
        for q0 in range(0, kn, QUAD):
            qn = min(QUAD, kn - q0)
            ent = []  # (b, ch, tl_min, tl_max)
            for b in range(NBANK):
                percore = seg[si * NBANK + b]
                # per chunk: union tile window
                win = {}
                for c in range(ncores):
                    for ki in range(q0, q0 + qn):
                        _, s0, s1 = percore[c][ki]
                        if s1 <= s0:
                            continue
                        for ch in range(s0 // 128, -(-s1 // 128)):
                            tl = ki - q0
                            if ch in win:
                                a, z = win[ch]
                                win[ch] = (min(a, tl), max(z, tl))
                            else:
                                win[ch] = (tl, tl)
                for ch in sorted(win):
                    a, z = win[ch]
                    ent.append((b, ch, a, z))
            if not ent:
                ent.append((0, 0, 0, qn - 1))  # dummy: zeroes the psum
            nb = len(ent)
            for j, (b, ch, a, z) in enumerate(ent):
                first = j == 0
                last = j == nb - 1
                if first or last:
                    a, z = 0, qn - 1  # full quad window
                bl.append((q0, int(gbase[si, b] + ch), n_cols, a, z - a + 1,
                           first, last))
                n_cols += 1
        builds.append(bl)

    # per-core arrays
    total_chunks = int(C.sum())
    idx_all = np.zeros((ncores, P, total_chunks * 8), np.int16)
    off_all = np.full((ncores, P, n_cols), 512.0, np.float32)
    # chunk column base per (si, b)
    cb = np.concatenate([[0], np.cumsum(C.reshape(-1))[:-1]]).reshape(
        n_sg, NBANK)
    for si, (k0, kn) in enumerate(sgs):
        for b in range(NBANK):
            nslot = int(C[si, b]) * 128
            if nslot == 0:
                continue
            percore = seg[si * NBANK + b]
            for c in range(ncores):
                rows = np.zeros(nslot, np.int64)
                offs = np.full(nslot, 512.0, np.float32)
                ktag = np.full(nslot, -1, np.int64)
                for (ki, s0, s1) in percore[c]:
                    if s1 == s0:
                        continue
                    te = tile_edges[c][k0 + ki]
                    e0, e1 = int(te[2][b]), int(te[2][b + 1])
                    rows[s0:s1] = te[0][e0:e1] - b * brows
                    offs[s0:s1] = te[1][e0:e1]
                    ktag[s0:s1] = ki
                blk = rows.astype(np.int16).reshape(-1, 16).T  # [16, n/16]
                c0 = int(cb[si, b])
                idx_all[c, :, c0 * 8: c0 * 8 + nslot // 16] = np.tile(
                    blk, (8, 1))
                # off columns for builds of this bank
                for (q0, gcol, col, a, w, _s, _e) in builds[si]:
                    ch = gcol - int(gbase[si, b])
                    if not (0 <= ch < int(C[si, b])):
                        continue
                    sl = slice(ch * 128, (ch + 1) * 128)
                    kt = ktag[sl]
                    inw = (kt >= q0 + a) & (kt < q0 + a + w)
                    off_all[c, :, col] = np.where(
                        inw, (kt - (q0 + a)) * 128 + offs[sl], 512.0)

    return dict(sgs=sgs, C=C, M=M, gbase=gbase, builds=builds, n_cols=n_cols,
                idx=idx_all, off=off_all, nseg=total_chunks)


def preprocess(feats, W, b, prelu_a, src_pos, dst_pos, src_neg, dst_neg,
               cfg: Config):
    n, ncores, tc = cfg.n_nodes, cfg.n_cores, cfg.t_core
    feats = np.asarray(feats, np.float32)
    W = np.asarray(W, np.float32)
    b = np.asarray(b, np.float32)
    prelu_a = np.asarray(prelu_a, np.float32)

    xs, plans, layouts, nds = [], [], [], []
    for src, dst in ((src_pos, dst_pos), (src_neg, dst_neg)):
        src = np.asarray(src, np.int64)
        dst = np.asarray(dst, np.int64)
        dego = np.bincount(src, minlength=n).astype(np.float64)
        degi = np.bincount(dst, minlength=n).astype(np.float64)
        ns = np.where(dego > 0, 1.0 / np.sqrt(np.maximum(dego, 1.0)), 0.0)
        nd = np.where(degi > 0, 1.0 / np.sqrt(np.maximum(degi, 1.0)), 0.0)
        x = np.zeros((cfg.n_pad, D), np.float16)
        x[:n] = (feats * ns[:, None].astype(np.float32)).astype(np.float16)
        xs.append(x)
        nds.append(nd.astype(np.float32))
        bank = src // cfg.bank_rows
        bcnt = np.zeros((cfg.t_global, NBANK), np.int64)
        np.add.at(bcnt, ((dst // P), bank), 1)
        ct = _deal_tiles(bcnt, cfg)
        plans.append(dict(core_tiles=ct))
        layouts.append(_layout_graph(src, dst, ct, cfg))

    # nd per (graph, position, core): [ncores, P, 2*t_core] f32
    nd_arr = np.zeros((ncores, P, 2 * tc), np.float32)
    for g in range(2):
        ndpad = np.zeros(cfg.n_pad, np.float32)
        ndpad[:n] = nds[g]
        ndt = ndpad.reshape(cfg.t_global, P).T
        ct = plans[g]["core_tiles"]
        for c in range(ncores):
            valid = ct[c] >= 0
            nd_arr[c][:, g * tc: (g + 1) * tc][:, valid] = ndt[:, ct[c][valid]]

    iota = np.tile(np.arange(QUAD * P, dtype=np.float32),
                   (P, 1)).astype(np.float16)
    a_rep = np.full((P, 1), float(prelu_a.reshape(-1)[0]), np.float32)
    b_rep = np.tile(b.reshape(1, D), (P, 1)).astype(np.float32)

    in_maps = []
    for c in range(ncores):
        in_maps.append({
            "x0": xs[0], "x1": xs[1],
            "w_in": W, "a_rep": a_rep, "b_rep": b_rep,
            "nd_in": nd_arr[c],
            "idx_in": np.concatenate(
                [layouts[0]["idx"][c], layouts[1]["idx"][c]], axis=1),
            "off_in": np.concatenate(
                [layouts[0]["off"][c], layouts[1]["off"][c]], axis=1),
            "iota_in": iota,
        })
    meta = {
        "layouts": layouts,
        "use_bias": bool(np.any(b != 0.0)),
    }
    return in_maps, plans, meta


# --------------------------------------------------------------------------
# Device kernel builder
# --------------------------------------------------------------------------
def build_kernel(nc, tc, cfg: Config, meta):
    from contextlib import ExitStack

    import concourse.mybir as mybir

    f32 = mybir.dt.float32
    f16 = mybir.dt.float16
    i16 = mybir.dt.int16
    Alu = mybir.AluOpType
    Act = mybir.ActivationFunctionType

    tcn, npad = cfg.t_core, cfg.n_pad
    layouts = meta["layouts"]
    use_bias = meta["use_bias"]
    nseg = [layouts[g]["nseg"] for g in range(2)]
    ncols = [layouts[g]["n_cols"] for g in range(2)]
    # max chunks/cols per supergroup (for fixed-size pool tiles)
    sg_chunks = []
    sg_cols = []
    for g in range(2):
        for si in range(len(layouts[g]["sgs"])):
            sg_chunks.append(int(layouts[g]["C"][si].sum()))
            sg_cols.append(len(layouts[g]["builds"][si]))
    cmax = max(sg_chunks)
    colmax = max(sg_cols)
    knmax = max(kn for g in range(2) for (_k0, kn) in layouts[g]["sgs"])

    x_dram = [nc.dram_tensor(f"x{g}", [npad, D], f16, kind="ExternalInput").ap()
              for g in range(2)]
    w_in = nc.dram_tensor("w_in", [P, D], f32, kind="ExternalInput").ap()
    a_rep = nc.dram_tensor("a_rep", [P, 1], f32, kind="ExternalInput").ap()
    b_rep = nc.dram_tensor("b_rep", [P, D], f32, kind="ExternalInput").ap()
    nd_in = nc.dram_tensor("nd_in", [P, 2 * tcn], f32, kind="ExternalInput").ap()
    idx_in = nc.dram_tensor("idx_in", [P, 8 * sum(nseg)], i16,
                            kind="ExternalInput").ap()
    off_in = nc.dram_tensor("off_in", [P, sum(ncols)], f32,
                            kind="ExternalInput").ap()
    iota_in = nc.dram_tensor("iota_in", [P, QUAD * P], f16,
                             kind="ExternalInput").ap()
    out = nc.dram_tensor("out", [2, P, tcn, D], f16, kind="ExternalOutput").ap()

    pb = dict(gpool=cfg.gbufs, ipool=2, opool=2, ohpool=32, apool=6,
              spool=2, tpool=4, ppool=4, hpool=4)
    pb.update(globals().get("POOL_BUFS") or {})

    with ExitStack() as ctx:
        const = ctx.enter_context(tc.tile_pool(name="const", bufs=1))
        gpool = ctx.enter_context(tc.tile_pool(name="gpool", bufs=pb["gpool"]))
        ipool = ctx.enter_context(tc.tile_pool(name="ipool", bufs=pb["ipool"]))
        opool = ctx.enter_context(tc.tile_pool(name="opool", bufs=pb["opool"]))
        ohpool = ctx.enter_context(tc.tile_pool(name="ohpool",
                                                bufs=pb["ohpool"]))
        apool = ctx.enter_context(tc.tile_pool(name="apool", bufs=pb["apool"]))
        spool = ctx.enter_context(tc.tile_pool(name="spool", bufs=pb["spool"]))
        tpool = ctx.enter_context(tc.tile_pool(name="tpool", bufs=pb["tpool"]))
        ppool = ctx.enter_context(tc.tile_pool(name="ppool", bufs=pb["ppool"],
                                               space="PSUM"))
        hpool = ctx.enter_context(tc.tile_pool(name="hpool", bufs=pb["hpool"],
                                               space="PSUM"))

        idx_base = [0, 8 * nseg[0]]
        col_base = [0, ncols[0]]
        # job list: interleave the two graphs' supergroups
        jobs = []
        for si in range(max(len(layouts[0]["sgs"]), len(layouts[1]["sgs"]))):
            for g in range(2):
                if si < len(layouts[g]["sgs"]):
                    jobs.append((g, si))

        # running chunk/col offsets per graph
        coff = [np.concatenate([[0], np.cumsum(
            layouts[g]["C"].reshape(-1))]).astype(int) for g in range(2)]
        boff = [np.concatenate([[0], np.cumsum(
            [len(bl) for bl in layouts[g]["builds"]])]).astype(int)
            for g in range(2)]
        # slot coverage written by each gather buffer's first-use job; later
        # jobs on the same buffer may use exact (unpadded) num_idxs only
        # where their region was fully pre-written (masked stale slots must
        # hold finite f16 data, not uninitialized SBUF).
        buf_cover = [0] * pb["gpool"]

        ipart = 16 if cfg.idx16 else P

        def issue_loads(jidx, g, si):
            lay = layouts[g]
            Crow = lay["C"][si]
            nch = int(Crow.sum())
            c0 = int(coff[g][si * NBANK])
            bl = lay["builds"][si]
            col0 = int(boff[g][si])
            it = ipool.tile([P, cmax * 8], i16, tag="gidx")
            nc.sync.dma_start(
                out=it[:ipart, : nch * 8],
                in_=idx_in[:ipart, idx_base[g] + c0 * 8:
                           idx_base[g] + (c0 + nch) * 8])
            ot = opool.tile([P, colmax], f32, tag="goff")
            nc.sync.dma_start(
                out=ot[:, : len(bl)],
                in_=off_in[:, col_base[g] + col0:
                           col_base[g] + col0 + len(bl)])
            gt = gpool.tile([P, cmax, D], f16, tag="gather")
            first_use = jidx < pb["gpool"]
            exact_ok = (cfg.exact_ni and not first_use
                        and nch <= buf_cover[jidx % pb["gpool"]])
            if first_use:
                buf_cover[jidx % pb["gpool"]] = nch
            for b in range(NBANK):
                Cb = int(Crow[b])
                if Cb == 0:
                    continue
                lo = int(lay["gbase"][si, b])
                rows = min(cfg.bank_rows, npad - b * cfg.bank_rows)
                ni = int(lay["M"][si, b]) if exact_ok else Cb * P
                head = cfg.ghead if (jidx == 0 and b == 0
                                     and Cb > 2 * cfg.ghead) else 0
                for (lo2, Cb2, ni2) in (
                        [(lo, head, head * P),
                         (lo + head, Cb - head, ni - head * P)]
                        if head else [(lo, Cb, ni)]):
                    icols = -(-ni2 // 16)
                    nc.gpsimd.dma_gather(
                        out_ap=gt[:, lo2: lo2 + Cb2, :],
                        in_ap=x_dram[g][b * cfg.bank_rows:
                                        b * cfg.bank_rows + rows, :],
                        idxs_ap=it[:, lo2 * 8: lo2 * 8 + icols],
                        num_idxs=ni2, num_idxs_reg=ni2,
                        elem_size=D, single_packet=False)
            return it, ot, gt

        if cfg.idx16:
            # idx DMAs only write partitions 0..15 (all the gather reads);
            # zero the idx buffers once so the rest is initialized.
            for _ in range(pb["ipool"]):
                zt = ipool.tile([P, cmax * 8], i16, tag="gidx")
                nc.vector.memset(zt[:], 0)

        # job 0's loads + gathers go first so the DMA stream starts
        # immediately; constants follow (first needed ~10us in).
        pre = {0: issue_loads(0, *jobs[0])}

        # ---- constants ----
        iota_sb = const.tile([P, QUAD * P], f16)
        nc.sync.dma_start(out=iota_sb[:], in_=iota_in)
        w_sb = const.tile([P, D], f16)
        nc.gpsimd.dma_start(out=w_sb[:], in_=w_in)  # f32 -> f16 cast DMA
        a_sb = const.tile([P, 1], f32)
        nc.sync.dma_start(out=a_sb[:], in_=a_rep)
        nd_sb = const.tile([P, 2 * tcn], f32)
        nc.sync.dma_start(out=nd_sb[:], in_=nd_in)
        if use_bias:
            b_sb = const.tile([P, D], f32)
            nc.sync.dma_start(out=b_sb[:], in_=b_rep)

        obuild = 0  # global one-hot build counter for engine assignment
        pool_cut = len(jobs) - cfg.pool_cut if cfg.pool_cut else len(jobs)
        for jidx, (g, si) in enumerate(jobs):
            lay = layouts[g]
            (k0, kn) = lay["sgs"][si]
            bl = lay["builds"][si]
            col0 = int(boff[g][si])
            it, ot, gt = pre.pop(jidx, None) or issue_loads(jidx, g, si)

            stg = spool.tile([P, knmax, D], f16, tag="stg")
            # group builds by quad
            by_q = {}
            for (q0, gcol, col, a, w, s, e) in bl:
                by_q.setdefault(q0, []).append((gcol, col, a, w, s, e))
            for q0 in sorted(by_q):
                qn = min(QUAD, kn - q0)
                ps = ppool.tile([P, QUAD * D], f32)
                for (gcol, col, a, w, s, e) in by_q[q0]:
                    oh = ohpool.tile([P, QUAD * P], f16)
                    eng = nc.vector
                    if (w == 1 and cfg.oh_mod and jidx < pool_cut
                            and (obuild % cfg.oh_mod == 0)):
                        eng = nc.gpsimd
                    obuild += 1
                    eng.tensor_scalar(
                        out=oh[:, : w * P], in0=iota_sb[:, : w * P],
                        scalar1=ot[:, col - col0: col - col0 + 1],
                        scalar2=None, op0=Alu.is_equal)
                    nc.tensor.matmul(
                        out=ps[:, a * D: (a + w) * D],
                        lhsT=gt[:, gcol, :], rhs=oh[:, : w * P],
                        start=s, stop=e)
                # aggT (psum, [d, quad*j]) -> SBUF f16, one copy per quad
                at = apool.tile([P, QUAD * D], f16, tag="aggT")
                nc.scalar.activation(out=at[:, : qn * D],
                                     in_=ps[:, : qn * D], func=Act.Copy)
                for ki in range(q0, q0 + qn):
                    tl = ki - q0
                    hp = hpool.tile([P, D], f32)
                    nc.tensor.matmul(out=hp[:], lhsT=at[:, tl * D: (tl + 1) * D],
                                     rhs=w_sb[:], start=True, stop=True)
                    kslot = g * tcn + (k0 + ki)
                    if cfg.act_prelu and not use_bias:
                        nc.scalar.activation(
                            out=stg[:, ki, :], in_=hp[:], func=Act.Prelu,
                            scale=nd_sb[:, kslot: kslot + 1],
                            alpha=a_sb[:, :1])
                    else:
                        h1 = tpool.tile([P, D], f32, tag="h1")
                        nc.vector.tensor_scalar(
                            out=h1[:], in0=hp[:],
                            scalar1=nd_sb[:, kslot: kslot + 1],
                            scalar2=None, op0=Alu.mult)
                        if use_bias:
                            h2 = tpool.tile([P, D], f32, tag="h2")
                            nc.vector.tensor_tensor(out=h2[:], in0=h1[:],
                                                    in1=b_sb[:], op=Alu.add)
                            h1 = h2
                        neg = tpool.tile([P, D], f32, tag="neg")
                        nc.vector.tensor_scalar(
                            out=neg[:], in0=h1[:], scalar1=0.0,
                            scalar2=a_sb[:, :1], op0=Alu.min, op1=Alu.mult)
                        pos = tpool.tile([P, D], f32, tag="pos")
                        nc.vector.tensor_scalar(
                            out=pos[:], in0=h1[:], scalar1=0.0,
                            scalar2=None, op0=Alu.max)
                        nc.vector.tensor_tensor(out=stg[:, ki, :], in0=neg[:],
                                                in1=pos[:], op=Alu.add)
            nc.sync.dma_start(out=out[g, :, k0: k0 + kn, :],
                              in_=stg[:, :kn, :])
    return out


# --------------------------------------------------------------------------
# Driver
# --------------------------------------------------------------------------
def _build_program(cfg: Config, meta):
    import concourse.bacc as bacc
    import concourse.tile as tile

    nc = bacc.Bacc("TRN2", target_bir_lowering=False, debug=False,
                   enable_asserts=False, num_devices=cfg.n_cores)
    with tile.TileContext(nc) as tc:
        build_kernel(nc, tc, cfg, meta)
    nc.compile()
    return nc


def _unscramble(results, plans, cfg: Config):
    n = cfg.n_nodes
    full = np.zeros((2, n, D), np.float32)
    for g in range(2):
        ct_all = plans[g]["core_tiles"]
        for core in range(cfg.n_cores):
            oc = np.asarray(results[core]["out"], np.float32)  # [2,P,tc,D]
            for k in range(cfg.t_core):
                t = int(ct_all[core, k])
                if t < 0:
                    continue
                r0 = t * P
                r1 = min(r0 + P, n)
                full[g, r0:r1] = oc[g, : r1 - r0, k, :]
    return full


_PROGRAM_CACHE = {}


def _meta_key(cfg: Config, meta):
    import hashlib
    hsh = hashlib.sha256()
    for g in range(2):
        lay = meta["layouts"][g]
        hsh.update(lay["C"].tobytes())
        for bl in lay["builds"]:
            hsh.update(np.asarray(bl, np.int64).tobytes())
    return (cfg.n_nodes, cfg.n_cores, cfg.sg, cfg.oh_mod, cfg.act_prelu,
            cfg.gbufs, cfg.idx16, cfg.exact_ni, cfg.pool_cut, cfg.ghead,
            meta["use_bias"],
            hsh.hexdigest())


def run(inputs, cfg: Config, trace=False):
    from concourse.bass_utils import run_bass_kernel_spmd

    in_maps, plans, meta = preprocess(
        inputs["feats"], inputs["W"], inputs["b"], inputs["prelu_a"],
        inputs["src_pos"], inputs["dst_pos"],
        inputs["src_neg"], inputs["dst_neg"], cfg)

    key = _meta_key(cfg, meta)
    nc = _PROGRAM_CACHE.get(key)
    if nc is None:
        nc = _build_program(cfg, meta)
        _PROGRAM_CACHE[key] = nc

    kwargs = {}
    if trace:
        kwargs = dict(trace=True, tmpdir=tempfile.mkdtemp(prefix="bgc_trace_"))
    try:
        res = run_bass_kernel_spmd(nc, in_maps,
                                   core_ids=list(range(cfg.n_cores)), **kwargs)
    except Exception:
        # Intermittent NRT_EXEC_UNIT_UNRECOVERABLE wedge under the axon
        # tunnel; a retry (with core reset requested) usually clears it.
        import os
        os.environ["NEURON_RT_RESET_CORES"] = "1"
        res = run_bass_kernel_spmd(nc, in_maps,
                                   core_ids=list(range(cfg.n_cores)), **kwargs)
    full = _unscramble(res.results, plans, cfg)
    return full, res


def kernel(**inputs) -> np.ndarray:
    cfg = Config()
    full, _ = run(inputs, cfg)
    return full
